# revision 2
# baseline (speedup 1.0000x reference)
"""Trainium2 Bass kernel for nn_Attention_73486890434886 (v2).

Gated 8-head attention (head_dim 32) with a full [8, 2048, 2048] attention
bias, batch 1, q_len = kv_len = 2048, fused QG / KV projections and a gated
output projection.

Strategy (8 NeuronCores, SPMD, no collectives):
  - Shard the 2048 q rows across the 8 cores (256 rows each); kv-side
    projections replicated.  All math in transposed orientation (no on-device
    transposes).
  - v3 changes vs baseline:
    * q/k side in fp8e4 with DoubleRow matmuls (0.5 cyc/row): qg projection,
      k projection, logits.  v side stays bf16 (precision).
    * logits contraction packs all 8 heads as 2 DoubleRow k-tiles.
    * bias injected into PSUM via identity-stationary matmul (DVE elementwise
      runs 1x on HW, so the PE is the cheapest venue), exp over both PSUM
      banks in one ACT call.
    * v bias folded into the gating tail (per-partition scalar).
    * reciprocal_approx_fast for softmax denominators.
    * kv projections + gate projections interleaved into the g=0 attention
      pass for pipelining.
"""

import numpy as np
import ml_dtypes

import concourse.bass as bass
import concourse.mybir as mybir
import concourse.tile as tile
from concourse import bacc
from concourse.bass_utils import run_bass_kernel_spmd

BF16 = ml_dtypes.bfloat16
F8 = ml_dtypes.float8_e4m3

# Problem shapes (hardcoded per the task statement).
B, QL, KVL, D, H, C, O = 1, 2048, 2048, 256, 8, 32, 256
NCORES = 8
QS = QL // NCORES          # 256 q rows per core
NKC = KVL // 128           # 16 kv chunks of 128
NG = 2                     # head groups (0-3, 4-7)
HPG = H // NG              # heads per group = 4

f32 = mybir.dt.float32
bf16 = mybir.dt.bfloat16
fp8 = mybir.dt.float8e4
u32 = mybir.dt.uint32
DR = mybir.MatmulPerfMode.DoubleRow

# pk8 column offsets (fp8 pack)
PK8_WQ = 0                     # [2, H, 128]   = 2048
PK8_WG = 2048                  # [2, 4, 128]   = 1024
PK8_WK = 3072                  # [2, NG, 128]  = 512
PK8_KVI = 3584                 # [2, KVL]      = 4096
PK8_QI = 7680                  # [2, QS]       = 512
PK8_N = 8192

# pk16 column offsets (bf16 pack)
PK16_WV = 0                    # [2, 256]      = 512
PK16_OW = 512                  # [4, 2, 128]   = 1024
PK16_IND2 = 1536               # [128]
PK16_IDEN = 1664               # [128]
PK16_KVI = 1792                # [2, KVL]      = 4096
PK16_N = 5888

# pk32 column offsets (f32 pack)
PK32_QB = 0                    # [H]    q bias (padded rows, *scale)
PK32_GB = 8                    # [4]    gate bias / 2 (bank layout)
PK32_KB = 12                   # [2]    k bias (group-packed rows)
PK32_VB = 14                   # [4]    v bias (bank layout rows)
PK32_OB = 18                   # [2]    o bias
PK32_N = 20


# ---------------------------------------------------------------------------
# Host-side packing
# ---------------------------------------------------------------------------

def _pack_shared(inputs):
    kv = np.asarray(inputs["kv_inputs"], np.float32)[0]        # [KVL, D]
    qg_w = np.asarray(inputs["qg_weights"], np.float32)[:, 0]  # [D, H, 2C]
    qg_b = np.asarray(inputs["qg_bias"], np.float32)[0, :, 0]  # [H, 2C]
    kv_w = np.asarray(inputs["kv_weights"], np.float32)[:, 0]  # [D, H, 2C]
    kv_b = np.asarray(inputs["kv_bias"], np.float32)[0, :, 0]  # [H, 2C]
    o_w = np.asarray(inputs["o_weights"], np.float32)[0]       # [H, C, O]
    o_b = np.asarray(inputs["o_bias"], np.float32)[:, 0]       # [O]

    scale = C ** -0.5

    # wq8 [128, 2(dtile), H, 128]: head h's w_q (*scale) at cols 32hp..+32,
    # zeros elsewhere; contraction d = dtile*128 + p.
    wq_full = qg_w[:, :, :C] * scale                           # [D, H, C]
    wq8 = np.zeros((128, 2, H, 128), np.float32)
    for h in range(H):
        hp = h % HPG
        for t in range(2):
            wq8[:, t, h, 32 * hp:32 * hp + 32] = wq_full[t * 128:(t + 1) * 128, h, :]

    # wg8 [128, 2, 4, 128]: bank gb=2g+b holds heads 4g+2b (cols 0:32) and
    # 4g+2b+1 (cols 64:96)  (matches acc-bank row layout).
    wg_full = qg_w[:, :, C:]                                   # [D, H, C]
    wg8 = np.zeros((128, 2, 4, 128), np.float32)
    gbn = np.zeros((128, 4), np.float32)                       # gate_bias / 2
    for g in range(NG):
        for b in range(2):
            gb = 2 * g + b
            for j in range(2):
                h = 4 * g + 2 * b + j
                for t in range(2):
                    wg8[:, t, gb, 64 * j:64 * j + C] = wg_full[t * 128:(t + 1) * 128, h, :]
                gbn[64 * j:64 * j + C, gb] = 0.5 * qg_b[h, C:]

    # wk8 [128, 2(dtile), NG, 128]: group t packs heads 4t..4t+3 at
    # cols hp*32..+32.
    wk_full = kv_w[:, :, :C]                                   # [D, H, C]
    wk8 = np.zeros((128, 2, NG, 128), np.float32)
    for t in range(NG):
        for hp in range(HPG):
            h = HPG * t + hp
            for dt_ in range(2):
                wk8[:, dt_, t, 32 * hp:32 * hp + 32] = wk_full[dt_ * 128:(dt_ + 1) * 128, h, :]

    # kviT8 / kviT [128, 2, KVL]: kv inputs transposed, d = dtile*128+p.
    kviT = kv.T.reshape(2, 128, KVL).transpose(1, 0, 2)        # [128, 2, KVL]

    # qiT8 is per-core.

    # wv bf16 [128, 2, 256]
    wv = kv_w[:, :, C:].reshape(D, H * C)
    wv = wv.reshape(2, 128, H * C).transpose(1, 0, 2)          # [128, 2, 256]

    # ow bf16 [128, 4, 2, 128], zero rows outside the two 32-row head blocks.
    ow = np.zeros((128, 4, 2, 128), np.float32)
    o_flat = o_w.reshape(H * C, O)
    for g in range(NG):
        for b in range(2):
            gb = 2 * g + b
            for j in range(2):
                h = 4 * g + 2 * b + j
                for t in range(2):
                    ow[64 * j:64 * j + C, gb, t, :] = \
                        o_flat[h * C:(h + 1) * C, t * 128:(t + 1) * 128]

    ind2 = np.zeros((128, 128), np.float32)    # row broadcast m <- 64*(m//64)+32
    for m in range(128):
        ind2[64 * (m // 64) + 32, m] = 1.0
    iden = np.eye(128, dtype=np.float32)

    # f32 biases
    qb_full = qg_b[:, :C] * scale
    qbp = np.zeros((128, H), np.float32)
    for h in range(H):
        hp = h % HPG
        qbp[32 * hp:32 * hp + 32, h] = qb_full[h]
    kb = np.zeros((128, 2), np.float32)
    for t in range(NG):
        for hp in range(HPG):
            h = HPG * t + hp
            kb[32 * hp:32 * hp + 32, t] = kv_b[h, :C]
    vbq = np.zeros((128, 4), np.float32)
    for g in range(NG):
        for b in range(2):
            gb = 2 * g + b
            for j in range(2):
                h = 4 * g + 2 * b + j
                vbq[64 * j:64 * j + C, gb] = kv_b[h, C:]
    ob = o_b.reshape(2, 128).T                 # [128, 2]

    pk8_shared = np.concatenate([
        wq8.reshape(128, -1), wg8.reshape(128, -1), wk8.reshape(128, -1),
        kviT.reshape(128, -1),
    ], axis=1)                                  # [128, 7680] (qiT8 appended per-core)
    pk16 = np.concatenate([
        wv.reshape(128, -1), ow.reshape(128, -1), ind2, iden,
        kviT.reshape(128, -1),
    ], axis=1)                                  # [128, 5888]
    pk32 = np.concatenate([qbp, gbn, kb, vbq, ob], axis=1)     # [128, 20]
    return {
        "pk8_shared": np.ascontiguousarray(pk8_shared).astype(F8),
        "pk16": np.ascontiguousarray(pk16).astype(BF16),
        "pk32": np.ascontiguousarray(pk32).astype(np.float32),
    }


def _pack_core(inputs, core, shared):
    qs = core * QS
    q = np.asarray(inputs["q_inputs"], np.float32)[0]          # [QL, D]
    bias = np.asarray(inputs["bias"], np.float32)[0]           # [H, QL, KVL]

    qiT = q[qs:qs + QS].T.reshape(2, 128, QS).transpose(1, 0, 2)   # [128,2,QS]
    pk8 = np.concatenate(
        [shared["pk8_shared"], np.ascontiguousarray(qiT).astype(F8).reshape(128, -1)],
        axis=1)                                                 # [128, 8192]

    b = bias[:, qs:qs + QS, :]                   # [H, QS, KVL]
    b = b.reshape(NG, HPG, QS, NKC, 128)         # [g, hp, q, c, p]
    b = b.transpose(4, 0, 3, 1, 2)               # [p, g, c, hp, q]
    bT = b.reshape(128, NG, NKC, HPG * QS)

    return {
        "pk8": np.ascontiguousarray(pk8),
        "bT": np.ascontiguousarray(bT).astype(BF16),
    }


def make_in_maps(inputs):
    shared = _pack_shared(inputs)
    maps = []
    for core in range(NCORES):
        m = {"pk16": shared["pk16"], "pk32": shared["pk32"]}
        m.update(_pack_core(inputs, core, shared))
        maps.append(m)
    return maps


def gather_output(results):
    out = np.empty((1, QL, O), np.float32)
    for core, res in enumerate(results):
        oT = np.asarray(res["out"], np.float32).reshape(O, QS)  # [o, q]
        out[0, core * QS:(core + 1) * QS, :] = oT.T
    return out


# ---------------------------------------------------------------------------
# Numpy mimic of the device dataflow (for host-side validation)
# ---------------------------------------------------------------------------

def _bf(x):
    return x.astype(BF16).astype(np.float32)


def _f8(x):
    return x.astype(F8).astype(np.float32)


def numpy_model(inputs):
    maps = make_in_maps(inputs)
    results = []
    for core in range(NCORES):
        m = maps[core]
        pk8 = np.asarray(m["pk8"], np.float32)
        pk16 = np.asarray(m["pk16"], np.float32)
        pk32 = np.asarray(m["pk32"], np.float32)
        bT = np.asarray(m["bT"], np.float32)      # [128, 2, 16, 1024]

        wq8 = pk8[:, PK8_WQ:PK8_WQ + 2048].reshape(128, 2, H, 128)
        wg8 = pk8[:, PK8_WG:PK8_WG + 1024].reshape(128, 2, 4, 128)
        wk8 = pk8[:, PK8_WK:PK8_WK + 512].reshape(128, 2, NG, 128)
        kvi8 = pk8[:, PK8_KVI:PK8_KVI + 4096].reshape(128, 2, KVL)
        qi8 = pk8[:, PK8_QI:PK8_QI + 512].reshape(128, 2, QS)
        wv = pk16[:, PK16_WV:PK16_WV + 512].reshape(128, 2, 256)
        ow = pk16[:, PK16_OW:PK16_OW + 1024].reshape(128, 4, 2, 128)
        ind2 = pk16[:, PK16_IND2:PK16_IND2 + 128]
        iden = pk16[:, PK16_IDEN:PK16_IDEN + 128]
        kvi = pk16[:, PK16_KVI:PK16_KVI + 4096].reshape(128, 2, KVL)
        qbp = pk32[:, PK32_QB:PK32_QB + 8]
        gbn = pk32[:, PK32_GB:PK32_GB + 4]
        kb = pk32[:, PK32_KB:PK32_KB + 2]
        vbq = pk32[:, PK32_VB:PK32_VB + 4]
        ob = pk32[:, PK32_OB:PK32_OB + 2]

        def dr(w, x):  # DoubleRow: sum_i w[:, i].T @ x[:, i]
            return w[:, 0].T @ x[:, 0] + w[:, 1].T @ x[:, 1]

        # qT8 [128, 2(ktile), H, QS] fp8, zero in the other ktile
        qT8 = np.zeros((128, 2, H, QS), np.float32)
        for h in range(H):
            g = h // HPG
            qT8[:, g, h, :] = _f8(dr(wq8[:, :, h, :], qi8) + qbp[:, h:h + 1])

        # sigT [128, 4, QS] f32
        sigT = np.zeros((128, 4, QS), np.float32)
        for gb in range(4):
            acc = dr(wg8[:, :, gb, :], qi8)
            t_u = np.tanh(0.5 * acc + gbn[:, gb:gb + 1])
            sigT[:, gb, :] = 0.5 * t_u + 0.5

        # kT8 [128, 2(group), KVL] fp8
        kT8 = np.zeros((128, 2, KVL), np.float32)
        for t in range(NG):
            for n in range(4):
                sl = slice(n * 512, (n + 1) * 512)
                kT8[:, t, sl] = _f8(dr(wk8[:, :, t, :], kvi8[:, :, sl]) + kb[:, t:t + 1])

        # v [128, NKC, H, 33] bf16 (no bias; ones col)
        vt = np.zeros((128, NKC, H, 33), np.float32)
        vt[:, :, :, 32] = 1.0
        for c in range(NKC):
            acc = np.zeros((128, H * C), np.float32)
            for kc in range(2):
                acc += kvi[:, kc, c * 128:(c + 1) * 128].T @ wv[:, kc, :]
            vt[:, c, :, :32] = _bf(acc).reshape(128, H, C)

        agT = np.zeros((128, 4, QS), np.float32)
        for g in range(NG):
            accb = [np.zeros((128, 512), np.float32) for _ in range(2)]
            for c in range(NKC):
                # logits: 2 DoubleRow calls (b2), stationary = kT8 chunk both groups
                kst = kT8[:, :, c * 128:(c + 1) * 128]       # [128, 2, 128]
                lt = np.zeros((128, HPG, QS), np.float32)
                for b2 in range(2):
                    h0 = HPG * g + 2 * b2
                    rhs = qT8[:, :, h0:h0 + 2, :].reshape(128, 2, 2 * QS)
                    lt[:, 2 * b2:2 * b2 + 2, :] = (
                        dr(kst, rhs).reshape(128, 2, QS)
                        + bT[:, g, c, 512 * b2:512 * (b2 + 1)].reshape(128, 2, QS))
                et = _bf(np.exp(lt))                          # ACT
                for hp in range(HPG):
                    h = HPG * g + hp
                    b2, j = hp // 2, hp % 2
                    accb[b2][64 * j:64 * j + 33, 0:QS] += vt[:, c, h, :].T @ et[:, hp, :]
            for b2 in range(2):
                gb = 2 * g + b2
                acc = accb[b2]
                rsg = np.zeros((128, QS), np.float32)
                rsg[32] = _bf(acc[32, 0:QS])
                rsg[96] = _bf(acc[96, 0:QS])
                rsb = ind2.T @ rsg
                with np.errstate(divide="ignore"):
                    recipB = 1.0 / rsb
                t1 = acc[:, 0:QS] * recipB
                agT[:, gb, :] = _bf((t1 + vbq[:, gb:gb + 1]) * sigT[:, gb, :])

        outT = np.zeros((2, 128, QS), np.float32)
        for t in range(2):
            acc = np.zeros((128, QS), np.float32)
            for gb in range(4):
                acc += ow[:, gb, t, :].T @ agT[:, gb, :]
            outT[t] = acc + ob[:, t:t + 1]
        results.append({"out": outT})
    return gather_output(results)


# ---------------------------------------------------------------------------
# Device kernel builder
# ---------------------------------------------------------------------------

def build_kernel():
    nc = bacc.Bacc("TRN2", target_bir_lowering=False, debug=False)

    p_pk8 = nc.declare_dram_parameter("pk8", [128, PK8_N], fp8, False)
    p_pk16 = nc.declare_dram_parameter("pk16", [128, PK16_N], bf16, False)
    p_pk32 = nc.declare_dram_parameter("pk32", [128, PK32_N], f32, False)
    p_bT = nc.declare_dram_parameter("bT", [128, NG, NKC, HPG * QS], bf16, False)
    p_out = nc.declare_dram_parameter("out", [2, 128, QS], f32, True)

    Exp = mybir.ActivationFunctionType.Exp
    Tanh = mybir.ActivationFunctionType.Tanh
    ADD = mybir.AluOpType.add
    MUL = mybir.AluOpType.mult

    with tile.TileContext(nc) as tc:
        with (
            tc.tile_pool(name="sb", bufs=1) as sb,
            tc.tile_pool(name="etp", bufs=3) as etp,
            tc.tile_pool(name="tmp", bufs=2) as tmp,
            tc.tile_pool(name="psw", bufs=2, space="PSUM") as psw,
            tc.tile_pool(name="psacc", bufs=2, space="PSUM") as psacc,
            tc.tile_pool(name="pslt", bufs=2, space="PSUM") as pslt,
        ):
            # ---- DMAs (FIFO on the SP queue: ordered by consumption) ----
            s_pk32 = sb.tile([128, PK32_N], f32)
            nc.sync.dma_start(out=s_pk32, in_=p_pk32[:])
            s_pk8 = sb.tile([128, PK8_N], fp8)
            nc.sync.dma_start(out=s_pk8, in_=p_pk8[:])
            s_pk16 = sb.tile([128, PK16_N], bf16)
            nc.sync.dma_start(out=s_pk16, in_=p_pk16[:])
            s_bT = sb.tile([128, NG, NKC, HPG * QS], bf16)
            for g in range(NG):
                for quarter in range(4):
                    c0 = quarter * (NKC // 4)
                    nc.sync.dma_start(
                        out=s_bT[:, g, c0:c0 + NKC // 4, :],
                        in_=p_bT[:, g, c0:c0 + NKC // 4, :],
                    )

            s_wq8 = s_pk8[:, PK8_WQ:PK8_WQ + 2048].rearrange(
                "p (t h m) -> p t h m", t=2, h=H)
            s_wg8 = s_pk8[:, PK8_WG:PK8_WG + 1024].rearrange(
                "p (t g m) -> p t g m", t=2, g=4)
            s_wk8 = s_pk8[:, PK8_WK:PK8_WK + 512].rearrange(
                "p (t g m) -> p t g m", t=2, g=NG)
            s_kvi8 = s_pk8[:, PK8_KVI:PK8_KVI + 4096].rearrange(
                "p (t n) -> p t n", t=2)
            s_qi8 = s_pk8[:, PK8_QI:PK8_QI + 512].rearrange(
                "p (t n) -> p t n", t=2)
            s_wv = s_pk16[:, PK16_WV:PK16_WV + 512].rearrange(
                "p (t n) -> p t n", t=2)
            s_ow = s_pk16[:, PK16_OW:PK16_OW + 1024].rearrange(
                "p (g t m) -> p g t m", g=4, t=2)
            s_ind2 = s_pk16[:, PK16_IND2:PK16_IND2 + 128]
            s_iden = s_pk16[:, PK16_IDEN:PK16_IDEN + 128]
            s_kvi = s_pk16[:, PK16_KVI:PK16_KVI + 4096].rearrange(
                "p (t n) -> p t n", t=2)
            s_qbp = s_pk32[:, PK32_QB:PK32_QB + 8]
            s_gbn = s_pk32[:, PK32_GB:PK32_GB + 4]
            s_kb = s_pk32[:, PK32_KB:PK32_KB + 2]
            s_vbq = s_pk32[:, PK32_VB:PK32_VB + 4]
            s_ob = s_pk32[:, PK32_OB:PK32_OB + 2]

            # ---- small constants / zeroed tiles ----
            s_zcol = sb.tile([1, 128], bf16)
            nc.vector.memset(s_zcol, 0.0)
            s_zrow = sb.tile([1, 512], bf16)
            nc.vector.memset(s_zrow, 0.0)

            # qT8 [128, 2(ktile), H, QS] fp8, zeroed (padding + other-ktile)
            s_qT8 = sb.tile([128, 2, H, QS], fp8)
            nc.vector.memset(s_qT8.rearrange("p a h q -> p (a h q)").bitcast(u32), 0)

            s_kT8 = sb.tile([128, 2, KVL], fp8)
            s_v = sb.tile([128, NKC, H, 33], bf16)
            nc.vector.memset(s_v[:, :, :, 32:33], 1.0)

            s_sigT = sb.tile([128, 4, QS], f32)
            s_agT = sb.tile([128, 4, QS], bf16)

            # ---------------- projection emit helpers ----------------
            def emit_qproj(h):
                pt = psw.tile([128, 512], f32, tag="work", name=f"q_ps_{h}")
                nc.tensor.matmul(pt[:, :QS], lhsT=s_wq8[:, :, h, :], rhs=s_qi8,
                                 start=True, stop=True, perf_mode=DR)
                g = h // HPG
                nc.vector.tensor_scalar_add(
                    s_qT8[:, g, h, :], pt[:, :QS], s_qbp[:, h:h + 1])

            def emit_gproj(gb):
                pt = psw.tile([128, 512], f32, tag="work", name=f"g_ps_{gb}")
                nc.tensor.matmul(pt[:, :QS], lhsT=s_wg8[:, :, gb, :], rhs=s_qi8,
                                 start=True, stop=True, perf_mode=DR)
                t_u = tmp.tile([128, QS], f32, tag="sigtmp", name=f"sig_u_{gb}")
                nc.scalar.activation(t_u, pt[:, :QS], Tanh,
                                     bias=s_gbn[:, gb:gb + 1], scale=0.5)
                nc.vector.tensor_scalar(s_sigT[:, gb, :], t_u, 0.5, 0.5, MUL, ADD)

            def emit_kproj(n):
                # both groups for kv cols n*512..(n+1)*512
                for t in range(NG):
                    pt = psw.tile([128, 512], f32, tag="work", name=f"k_ps_{t}_{n}")
                    nc.tensor.matmul(
                        pt, lhsT=s_wk8[:, :, t, :],
                        rhs=s_kvi8[:, :, n * 512:(n + 1) * 512],
                        start=True, stop=True, perf_mode=DR)
                    nc.vector.tensor_scalar_add(
                        s_kT8[:, t, n * 512:(n + 1) * 512], pt, s_kb[:, t:t + 1])

            def emit_vproj(c):
                pt = psw.tile([128, 512], f32, tag="work", name=f"v_ps_{c}")
                for kc in range(2):
                    nc.tensor.matmul(
                        pt[:, :256], lhsT=s_kvi[:, kc, c * 128:(c + 1) * 128],
                        rhs=s_wv[:, kc, :], start=(kc == 0), stop=(kc == 1))
                nc.vector.tensor_copy(
                    out=s_v[:, c, :, 0:32],
                    in_=pt[:, :256].rearrange("p (h x) -> p h x", h=H))

            # ---------------- bootstrap projections ----------------
            for h in range(HPG):        # q heads 0-3 (needed first, g=0)
                emit_qproj(h)
            emit_kproj(0)               # kv chunks 0-3
            emit_vproj(0)
            emit_vproj(1)

            # interleave schedule for the g=0 pass:
            interleave = {
                0: lambda: [emit_qproj(4), emit_qproj(5)],
                1: lambda: [emit_qproj(6), emit_qproj(7), emit_kproj(1)],
                2: lambda: [emit_gproj(0), emit_gproj(1)],
                3: lambda: [emit_gproj(2), emit_gproj(3)],
                4: lambda: [emit_kproj(2)],
                8: lambda: [emit_kproj(3)],
            }

            # ---------------- attention ----------------
            for g in range(NG):
                accs = []
                for b2 in range(2):
                    acc = psacc.tile([128, 512], f32, tag="accum",
                                     name=f"acc_{g}_{b2}")
                    nc.tensor.matmul(acc, lhsT=s_zcol, rhs=s_zrow, start=True,
                                     stop=False, skip_group_check=True)
                    accs.append(acc)
                for c in range(NKC):
                    lt = pslt.tile([128, HPG, QS], f32, tag="lt",
                                   name=f"lt_{g}_{c}")
                    kst = s_kT8[:, :, c * 128:(c + 1) * 128]
                    for b2 in range(2):
                        h0 = HPG * g + 2 * b2
                        nc.tensor.matmul(
                            lt[:, 2 * b2:2 * b2 + 2, :], lhsT=kst,
                            rhs=s_qT8[:, :, h0:h0 + 2, :],
                            start=True, stop=False, perf_mode=DR,
                            skip_group_check=True)
                        nc.tensor.matmul(
                            lt[:, 2 * b2:2 * b2 + 2, :], lhsT=s_iden,
                            rhs=s_bT[:, g, c, 512 * b2:512 * (b2 + 1)],
                            start=False, stop=True, skip_group_check=True)
                    et = etp.tile([128, HPG, QS], bf16, tag="et",
                                  name=f"et_{g}_{c}")
                    nc.scalar.activation(
                        et.rearrange("p a q -> p (a q)"),
                        lt.rearrange("p a q -> p (a q)"), Exp)
                    for hp in range(HPG):
                        h = HPG * g + hp
                        b2, j = hp // 2, hp % 2
                        nc.tensor.matmul(
                            accs[b2][64 * j:64 * j + 33, 0:QS],
                            lhsT=s_v[:, c, h, :], rhs=et[:, hp, :],
                            start=False, stop=(c == NKC - 1),
                            tile_position=(0, 64 * j), skip_group_check=True)
                    if g == 0 and c in interleave:
                        interleave[c]()
                    if g == 0 and c + 2 < NKC:
                        emit_vproj(c + 2)

                # ---- softmax denominator + gating per bank ----
                for b2 in range(2):
                    gb = 2 * g + b2
                    acc = accs[b2]
                    rsg = tmp.tile([128, QS], bf16, tag="rsg", name=f"rsg_{gb}")
                    nc.vector.memset(rsg.bitcast(u32), 0)
                    nc.vector.tensor_copy(out=rsg[32:33, :], in_=acc[32:33, 0:QS])
                    nc.vector.tensor_copy(out=rsg[96:97, :], in_=acc[96:97, 0:QS])
                    rsb = psw.tile([128, 512], f32, tag="work", name=f"rsb_{gb}")
                    nc.tensor.matmul(rsb[:, :QS], lhsT=s_ind2, rhs=rsg,
                                     start=True, stop=True)
                    recipB = tmp.tile([128, QS], f32, tag="recip",
                                      name=f"recip_{gb}")
                    nc.vector.reciprocal_approx_fast(out=recipB, in_=rsb[:, :QS])
                    t1 = tmp.tile([128, QS], f32, tag="gt1", name=f"gt1_{gb}")
                    nc.vector.scalar_tensor_tensor(
                        out=t1, in0=acc[:, 0:QS], scalar=1.0, in1=recipB,
                        op0=MUL, op1=MUL)
                    nc.vector.scalar_tensor_tensor(
                        out=s_agT[:, gb, :], in0=t1, scalar=s_vbq[:, gb:gb + 1],
                        in1=s_sigT[:, gb, :], op0=ADD, op1=MUL)

            # ---- output projection ----
            s_outT = sb.tile([128, 2, QS], f32)
            for t in range(2):
                pt = psw.tile([128, 512], f32, tag="work", name=f"o_ps_{t}")
                for gb in range(4):
                    nc.tensor.matmul(
                        pt[:, :QS], lhsT=s_ow[:, gb, t, :], rhs=s_agT[:, gb, :],
                        start=(gb == 0), stop=(gb == 3))
                nc.vector.tensor_scalar_add(
                    s_outT[:, t, :], pt[:, :QS], s_ob[:, t:t + 1])
            nc.sync.dma_start(
                out=p_out.rearrange("t p q -> p t q"), in_=s_outT)

    nc.finalize()
    return nc


_NC = None


def _get_nc():
    global _NC
    if _NC is None:
        _NC = build_kernel()
    return _NC


def kernel(**inputs) -> np.ndarray:
    nc = _get_nc()
    in_maps = make_in_maps(inputs)
    res = run_bass_kernel_spmd(nc, in_maps, core_ids=list(range(NCORES)))
    return gather_output(res.results)


def kernel_traced(**inputs):
    """Like kernel() but with NTFF profiling; returns (output, exec_time_ns, res)."""
    nc = _get_nc()
    in_maps = make_in_maps(inputs)
    res = run_bass_kernel_spmd(nc, in_maps, core_ids=list(range(NCORES)), trace=True)
    return gather_output(res.results), res.exec_time_ns, res
